# revision 1
# baseline (speedup 1.0000x reference)
"""Trainium2 Bass kernel for nn_BoxCrossCategoryLoss (8-core data-parallel).

Math: the reference loss is, per row,
    sum over 36 terms of relu(pAB[i][:,f1] + pBC[j][:,f2] - c)
where c is either pAC[k][:,1] (14 LOSS terms) or log1mexp(pAC[k][:,0])
(22 NEG terms), and p* = create_probabilities(log-volumes).  The three
int *_rel_id inputs are unused by the reference, so they are never
uploaded.

Decomposition used on-chip (per core, rows laid out as [128, NF] bf16):
  e = Exp(v)                      (ACT, fp32)
  l = Ln(1 - e)                   (ACT, scale=-1 bias=1, bf16 out)
  p-values   = v+l / l+v / v+v / l+l      (DVE tensor_tensor, bf16 2x)
  L_k = Ln(1 - P_k),  P_k = products of e / (1-e)  (DVE muls, ACT Ln)
  S = pAB + pBC               (14 sums, DVE bf16 2x)
  d = S - c                   (36 subs, DVE bf16 2x)
  relu+reduce: tensor_scalar(max,0)+accum_out (DVE 4x) or
               activation(Relu)+accum_out (ACT), split for engine balance.
Per-partition partial sums land in fp32 stats tiles, DMA'd out and
summed on host in float64.  bf16 end-to-end rel err ~4e-5 (validated).
"""

import os
import sys

import numpy as np

for _p in ("/opt/trn_rl_repo", "/root/.axon_site/_ro/trn_rl_repo"):
    if os.path.isdir(_p) and _p not in sys.path:
        sys.path.insert(0, _p)

import ml_dtypes  # noqa: E402
import concourse.bacc as bacc  # noqa: E402
from concourse import mybir, tile  # noqa: E402
from concourse.bass_utils import run_bass_kernel_spmd  # noqa: E402

BF16 = ml_dtypes.bfloat16
F32 = mybir.dt.float32
BF = mybir.dt.bfloat16
Alu = mybir.AluOpType
Act = mybir.ActivationFunctionType

N_CORES = 8
P = 128

PAIR_NAMES = ["AB", "BA", "BC", "CB", "AC", "CA"]
# Padding rows must contribute exactly zero loss: very negative AB/BC
# volumes make every S ~ -40 while c stays <= ~0, so relu(S-c) == 0.
PAD_VAL = {"AB": -20.0, "BA": -20.0, "BC": -20.0, "CB": -20.0,
           "AC": -1e-3, "CA": -1e-3}

# S_i = A[a] + B[b]  with  X[k,c] = pX[k][:, c]
S_DEFS = [
    ((0, 0), (0, 1)), ((0, 0), (2, 1)), ((1, 0), (1, 1)), ((1, 0), (2, 1)),
    ((2, 0), (0, 1)), ((2, 0), (1, 1)), ((2, 0), (2, 1)), ((2, 0), (3, 1)),
    ((0, 1), (0, 0)), ((0, 1), (2, 0)), ((1, 1), (1, 0)), ((1, 1), (2, 0)),
    ((2, 1), (2, 0)), ((3, 1), (2, 0)),
]
# 36 terms: (S index, c name);  Ck1 = pAC[k][:,1], Lk = log1mexp(pAC[k][:,0])
TERMS = [
    (0, "C01"), (1, "C01"), (2, "C11"), (3, "C11"), (4, "C01"), (5, "C11"),
    (6, "C21"), (7, "C31"), (8, "C01"), (9, "C01"), (10, "C11"), (11, "C11"),
    (12, "C21"), (13, "C31"),
    (0, "L1"), (0, "L2"), (1, "L1"), (1, "L2"), (2, "L0"), (2, "L2"),
    (3, "L0"), (3, "L2"), (4, "L1"), (4, "L2"), (5, "L0"), (5, "L2"),
    (8, "L1"), (8, "L2"), (9, "L1"), (9, "L2"), (10, "L0"), (10, "L2"),
    (11, "L0"), (11, "L2"), (7, "L2"), (13, "L2"),
]
# Engine split for the fused relu+reduce, assigned per S-group so the
# 1-3 terms of one S batch into a single slab op.  ACT groups carry 20
# term-passes, DVE groups 16 (balances the two engines).
# Cost-model sweep: ACT 15/20/26 term-passes -> 473/464/473 us; 20 is
# the balanced optimum on TRN2 (DVE relu 4x vs ACT relu 1x rates).
ACT_GROUPS = {0, 1, 4, 8, 9, 2, 7}     # S1,S2,S5,S9,S10,S3,S8 -> 20 passes
N_ACT = len(ACT_GROUPS)                 # 7 relu slots/chunk on ACT
N_DVE = 14 - N_ACT                      # 7 on DVE (16 term-passes)

A_SLOTS = [(0, 0), (1, 0), (2, 0), (0, 1), (1, 1), (2, 1), (3, 1)]


def make_chunks(nf: int) -> list[int]:
    chunks = [1344] * (nf // 1344)
    rem = nf - 1344 * len(chunks)
    if rem:
        chunks.append(rem)
    assert sum(chunks) == nf and all(c % 2 == 0 for c in chunks)
    return chunks


def _emit_pvals(nc, pool, tag_pre, F, v0, l0, v1, l1, slots):
    """p-values for one tensor pair; v*/l* keyed by col."""
    out = {}
    for (k, c) in slots:
        t = pool.tile([P, F], BF, tag=f"{tag_pre}{k}{c}")
        vv1, ll1 = v0[c], l0[c]
        vv2, ll2 = v1[c], l1[c]
        if k == 0:
            nc.vector.tensor_tensor(t[:], vv1[:], ll2[:], Alu.add)
        elif k == 1:
            nc.vector.tensor_tensor(t[:], ll1[:], vv2[:], Alu.add)
        elif k == 2:
            nc.vector.tensor_tensor(t[:], vv1[:], vv2[:], Alu.add)
        else:
            nc.vector.tensor_tensor(t[:], ll1[:], ll2[:], Alu.add)
        out[(k, c)] = t
    return out


def build_module(nf: int, chunks: list[int]):
    nchunks = len(chunks)
    nc = bacc.Bacc("TRN2", target_bir_lowering=False, debug=False,
                   enable_asserts=False, num_devices=N_CORES)
    in_aps = {}
    for X in PAIR_NAMES:
        for c in (0, 1):
            h = nc.dram_tensor(f"v_{X}{c}", [P, nf], BF, kind="ExternalInput")
            in_aps[(X, c)] = h.ap()
    out_v = nc.dram_tensor("stats_v", [P, N_DVE * nchunks], F32,
                           kind="ExternalOutput").ap()
    out_a = nc.dram_tensor("stats_a", [P, N_ACT * nchunks], F32,
                           kind="ExternalOutput").ap()

    terms_of = {}
    for ti, (si, cn) in enumerate(TERMS):
        terms_of.setdefault(si, []).append((ti, cn))

    from contextlib import ExitStack
    with tile.TileContext(nc) as tc, ExitStack() as ctx:
        vp = ctx.enter_context(tc.tile_pool(name="vp", bufs=1))
        ep = ctx.enter_context(tc.tile_pool(name="ep", bufs=4))
        lp = ctx.enter_context(tc.tile_pool(name="lp", bufs=5))
        mp = ctx.enter_context(tc.tile_pool(name="mp", bufs=3))
        pv = ctx.enter_context(tc.tile_pool(name="pv", bufs=1))
        sp = ctx.enter_context(tc.tile_pool(name="sp", bufs=4))
        Lp = ctx.enter_context(tc.tile_pool(name="Lp", bufs=1))
        dp = ctx.enter_context(tc.tile_pool(name="dp", bufs=2))
        stp = ctx.enter_context(tc.tile_pool(name="st", bufs=1))

        stats_v = stp.tile([P, N_DVE * nchunks], F32, tag="stv")
        stats_a = stp.tile([P, N_ACT * nchunks], F32, tag="sta")

        f0 = 0
        for k, F in enumerate(chunks):
            # ---- load all 12 column tiles for this chunk (AC pair first)
            v = {}
            for X in ["AC", "CA", "AB", "BA", "BC", "CB"]:
                for c in (0, 1):
                    t = vp.tile([P, F], BF, tag=f"v{X}{c}")
                    nc.sync.dma_start(t[:], in_aps[(X, c)][:, f0:f0 + F])
                    v[(X, c)] = t

            def exp_of(X, c):
                e = ep.tile([P, F], F32, tag="e")
                nc.scalar.activation(e[:], v[(X, c)][:], Act.Exp)
                return e

            def ln1m(src, dst_tag, pool):
                t = pool.tile([P, F], BF, tag=dst_tag)
                nc.scalar.activation(t[:], src[:], Act.Ln, bias=1.0, scale=-1.0)
                return t

            # ---- pair AC: e, l (col1 only), m/P products, L_k, C p-values
            eAC0, eCA0 = exp_of("AC", 0), exp_of("CA", 0)
            eAC1, eCA1 = exp_of("AC", 1), exp_of("CA", 1)
            lAC1 = ln1m(eAC1, "l", lp)
            lCA1 = ln1m(eCA1, "l", lp)
            mAC = mp.tile([P, F], F32, tag="mP")
            nc.vector.tensor_scalar(mAC[:], eAC0[:], -1.0, 1.0, Alu.mult, Alu.add)
            mCA = mp.tile([P, F], F32, tag="mP")
            nc.vector.tensor_scalar(mCA[:], eCA0[:], -1.0, 1.0, Alu.mult, Alu.add)
            L = {}
            for j, (x, y) in enumerate([(eAC0, mCA), (mAC, eCA0), (eAC0, eCA0)]):
                Pj = mp.tile([P, F], F32, tag="mP")
                nc.vector.tensor_tensor(Pj[:], x[:], y[:], Alu.mult)
                L[f"L{j}"] = ln1m(Pj, f"L{j}", Lp)
            Cvals = _emit_pvals(nc, pv, "C", F,
                                {1: v[("AC", 1)]}, {1: lAC1},
                                {1: v[("CA", 1)]}, {1: lCA1},
                                [(0, 1), (1, 1), (2, 1), (3, 1)])

            # ---- pair AB -> A p-values
            eAB0, eBA0 = exp_of("AB", 0), exp_of("BA", 0)
            eAB1, eBA1 = exp_of("AB", 1), exp_of("BA", 1)
            lAB = {0: ln1m(eAB0, "l", lp), 1: ln1m(eAB1, "l", lp)}
            lBA = {0: ln1m(eBA0, "l", lp), 1: ln1m(eBA1, "l", lp)}
            Avals = _emit_pvals(nc, pv, "A", F,
                                {0: v[("AB", 0)], 1: v[("AB", 1)]}, lAB,
                                {0: v[("BA", 0)], 1: v[("BA", 1)]}, lBA,
                                A_SLOTS)

            # ---- pair BC -> B p-values
            eBC0, eCB0 = exp_of("BC", 0), exp_of("CB", 0)
            eBC1, eCB1 = exp_of("BC", 1), exp_of("CB", 1)
            lBC = {0: ln1m(eBC0, "l", lp), 1: ln1m(eBC1, "l", lp)}
            lCB = {0: ln1m(eCB0, "l", lp), 1: ln1m(eCB1, "l", lp)}
            Bvals = _emit_pvals(nc, pv, "B", F,
                                {0: v[("BC", 0)], 1: v[("BC", 1)]}, lBC,
                                {0: v[("CB", 0)], 1: v[("CB", 1)]}, lCB,
                                A_SLOTS)

            cmap = {f"C{kk}1": Cvals[(kk, 1)] for kk in range(4)}
            cmap.update(L)
            # TERMS c-names use "C01" == pAC[0][:,1]
            cmap = {"C01": cmap["C01"], "C11": cmap["C11"],
                    "C21": cmap["C21"], "C31": cmap["C31"],
                    "L0": cmap["L0"], "L1": cmap["L1"], "L2": cmap["L2"]}

            # ---- S sums + 36 terms; the subs of one S write a contiguous
            # d-slab so its relu+reduce is ONE fused op over the slab.
            na = nv = 0
            for si, (a, b) in enumerate(S_DEFS):
                S = sp.tile([P, F], BF, tag="S")
                nc.vector.tensor_tensor(S[:], Avals[a][:], Bvals[b][:], Alu.add)
                terms = terms_of[si]
                nt = len(terms)
                d = dp.tile([P, nt * F], BF, tag="d")
                for j, (ti, cn) in enumerate(terms):
                    nc.vector.tensor_tensor(d[:, j * F:(j + 1) * F], S[:],
                                            cmap[cn][:], Alu.subtract)
                r = dp.tile([P, nt * F], BF, tag="r")
                if si in ACT_GROUPS:
                    slot = stats_a[:, k * N_ACT + na: k * N_ACT + na + 1]
                    nc.scalar.activation(r[:], d[:], Act.Relu, accum_out=slot)
                    na += 1
                else:
                    slot = stats_v[:, k * N_DVE + nv: k * N_DVE + nv + 1]
                    nc.vector.tensor_scalar(r[:], d[:], 0.0, None, Alu.max,
                                            Alu.add, accum_out=slot)
                    nv += 1
            assert na == N_ACT and nv == N_DVE
            f0 += F

        nc.sync.dma_start(out_v, stats_v[:])
        nc.sync.dma_start(out_a, stats_a[:])

    nc.compile()
    return nc


_CACHE = {}


def _get_module(nf, chunks):
    key = (nf, tuple(chunks))
    if key not in _CACHE:
        _CACHE[key] = build_module(nf, chunks)
    return _CACHE[key]


LAST_RESULTS = None  # BassKernelResults of the most recent run (for profiling)


def kernel(**inputs) -> np.ndarray:
    global LAST_RESULTS
    vols = {X: np.asarray(inputs["vol_" + X]) for X in PAIR_NAMES}
    n_rows = vols["AB"].shape[0]
    # rows per core laid out [128, nf]; nf even for DVE packed modes
    nf = -(-n_rows // (N_CORES * P))
    nf += nf % 2
    nf = max(nf, 160)
    # round up so chunking stays regular (multiples of 32 keep DMA tidy)
    nf = -(-nf // 32) * 32
    chunks = make_chunks(nf)
    total_rows = N_CORES * P * nf

    in_maps = [dict() for _ in range(N_CORES)]
    for X in PAIR_NAMES:
        a = vols[X].astype(np.float32, copy=False)
        for c in (0, 1):
            col = np.full(total_rows, PAD_VAL[X], dtype=np.float32)
            col[:n_rows] = a[:, c]
            colb = col.astype(BF16).reshape(N_CORES, P, nf)
            for core in range(N_CORES):
                in_maps[core][f"v_{X}{c}"] = np.ascontiguousarray(colb[core])

    nc = _get_module(nf, chunks)
    # NTFF tracing needs antenv.axon_hooks, absent in most axon client
    # environments; force it off so a stray BASS_TRACE can't crash the run.
    trace = bool(os.environ.get("BASS_TRACE"))
    if trace:
        try:
            from antenv import axon_hooks  # noqa: F401
        except ImportError:
            trace = False
    if not trace:
        os.environ["BASS_NEVER_TRACE"] = "1"
    res = run_bass_kernel_spmd(nc, in_maps, core_ids=list(range(N_CORES)),
                               trace=trace)
    LAST_RESULTS = res
    total = np.float64(0.0)
    for om in res.results:
        total += om["stats_v"].astype(np.float64).sum()
        total += om["stats_a"].astype(np.float64).sum()
    return np.asarray(total, dtype=np.float32)


if __name__ == "__main__":
    # quick smoke test on small random data
    rng = np.random.default_rng(0)
    n = 100_000
    ins = {}
    for X in PAIR_NAMES:
        u = rng.uniform(1e-6, 1 - 1e-6, size=(n, 2)).astype(np.float32)
        ins["vol_" + X] = np.log(u)
    for nm in ("xy_rel_id", "yz_rel_id", "xz_rel_id"):
        ins[nm] = rng.integers(0, 2, size=(n, 2)).astype(np.int32)
    print("kernel:", kernel(**ins))

